# revision 1
# baseline (speedup 1.0000x reference)
"""Trainium2 Bass kernel for nn_CombinedMLPMoEModel (moe_routing).

Strategy (8 NeuronCores, pure data parallel on the batch):
 - Host: shard batch 16384 -> 8 x 2048 tokens, pre-transpose x1/x2/x3 to
   feature-major [Din, tok] so every layer's activation sits with its
   contracted dim on SBUF partitions; replicate weights.
 - On chip, everything stays feature-major: out_fm = W.T @ act_fm with
   lhsT = W exactly as stored [Din, Dout].
 - Precision: the routing decision (top-2 of 8 experts) matches the fp32
   reference only if the MLP chain + router logits carry fp32 accuracy.
   Plain fp32 matmul costs 4 cyc/row on the PE; instead the MLP chain
   uses a 3-term fp32r (tf32-like, 1 cyc/row) split:
       W @ x ~= Whi@xhi + Whi@xlo + Wlo@xhi   (error ~1e-7, 3 cyc/row)
   with Whi/Wlo pre-split on the host and xhi/xlo split on chip.
   Expert matmuls + the folded output layer run in bf16 (routing indices
   stay exact; output rel-err ~3e-3).  LayerNorm statistics use single
   fp32r (their error is a per-token scale/shift, routing-order safe).
 - MoE: dense compute of all 8 experts per 512-token megatile in
   token-major PSUM [128 tok, 512], combined with per-token top-2
   softmax weights via scalar_tensor_tensor, then transposed back to
   feature-major on the PE.
 - The tail (concat(o) @ Wf -> bn -> @ Wr) is linear, so it folds on the
   host into one vector: out = concat(o) @ (Wf @ (scf * Wr)) + c0.
"""

import numpy as np
import ml_dtypes
from contextlib import ExitStack

import concourse.bass as bass
from concourse import bacc
import concourse.mybir as mybir
import concourse.tile as tile
from concourse.bass_utils import run_bass_kernel_spmd

F32 = mybir.dt.float32
F32R = mybir.dt.float32r
BF16 = mybir.dt.bfloat16
AF = mybir.ActivationFunctionType
ALU = mybir.AluOpType
AX = mybir.AxisListType

N_CORES = 8
B = 16384
DIN = 1024
D = 512
H = 1024
D3 = 3 * D          # 1536
E = 8
TOK_CORE = B // N_CORES   # 2048
MT = 512                  # megatile tokens
EPS = 1e-5
NEG_BIG = -1.0e30

_PROGRAM_CACHE = {}


def _chunks(n):
    return n // 128


def build_program(n_tok=TOK_CORE, mt=MT):
    """Build the per-core Bass program (SPMD across the 8 cores)."""
    nc = bacc.Bacc(None, target_bir_lowering=False)
    T = n_tok // mt
    CS = mt
    NC_TOK = _chunks(mt)

    # ---------------- DRAM I/O ----------------
    xs = [nc.dram_tensor(f"x{i+1}t", [DIN, n_tok], F32, kind="ExternalInput")
          for i in range(3)]
    Wp = [nc.dram_tensor(f"Wp{i+1}", [DIN, D], F32, kind="ExternalInput")
          for i in range(3)]
    W1 = nc.dram_tensor("W1", [D3, H], F32, kind="ExternalInput")
    W2 = nc.dram_tensor("W2", [H, H], F32, kind="ExternalInput")
    W3 = nc.dram_tensor("W3", [H, D3], F32, kind="ExternalInput")
    # packed per-feature vectors, [128, chunks] layout
    bp = [nc.dram_tensor(f"bp{i+1}", [128, _chunks(D)], F32, kind="ExternalInput")
          for i in range(3)]
    sc1 = nc.dram_tensor("sc1", [128, _chunks(H)], F32, kind="ExternalInput")
    bi1 = nc.dram_tensor("bi1", [128, _chunks(H)], F32, kind="ExternalInput")
    sc2 = nc.dram_tensor("sc2", [128, _chunks(H)], F32, kind="ExternalInput")
    bi2 = nc.dram_tensor("bi2", [128, _chunks(H)], F32, kind="ExternalInput")
    b3v = nc.dram_tensor("b3v", [128, _chunks(D3)], F32, kind="ExternalInput")
    lngv = nc.dram_tensor("lngv", [128, _chunks(D3)], F32, kind="ExternalInput")
    lnbv = nc.dram_tensor("lnbv", [128, _chunks(D3)], F32, kind="ExternalInput")
    Wg_d = nc.dram_tensor("Wg_r", [128, _chunks(D), E], F32, kind="ExternalInput")
    Wfr_d = nc.dram_tensor("Wfr_r", [128, _chunks(D3), 1], BF16, kind="ExternalInput")
    We_d = nc.dram_tensor("We_r", [128, E, _chunks(D), D], BF16, kind="ExternalInput")
    bexp_d = nc.dram_tensor("bexp_bf", [E, D], BF16, kind="ExternalInput")
    bg_d = nc.dram_tensor("bg_v", [1, E], F32, kind="ExternalInput")
    c0_d = nc.dram_tensor("c0_v", [1, 1], F32, kind="ExternalInput")
    ones_d = nc.dram_tensor("ones_col", [128, 1], F32, kind="ExternalInput")
    onesr_d = nc.dram_tensor("ones_row", [1, 128], F32, kind="ExternalInput")
    ident_d = nc.dram_tensor("ident", [128, 128], F32, kind="ExternalInput")
    out_d = nc.dram_tensor("out", [1, n_tok], F32, kind="ExternalOutput")

    with tile.TileContext(nc) as tc, ExitStack() as ctx:
        cp = ctx.enter_context(tc.tile_pool(name="consts", bufs=1))
        sp = ctx.enter_context(tc.tile_pool(name="work", bufs=1))
        ps = ctx.enter_context(tc.tile_pool(name="psum", bufs=8, space="PSUM"))

        def pt(shape, dtype, tag, bufs=None):
            return sp.tile(shape, dtype, tag=tag, bufs=bufs, name=tag)

        def mmtile(name="p"):
            return ps.tile([128, 512], F32, tag="mm", name=name)

        # ---------------- resident constants ----------------
        We_sb = cp.tile([128, E, _chunks(D), D], BF16, name="We_sb")
        Wg_sb = cp.tile([128, _chunks(D), E], F32, name="Wg_sb")
        nc.sync.dma_start(out=Wg_sb, in_=Wg_d[:, :, :])
        Wfr_sb = cp.tile([128, _chunks(D3), 1], BF16, name="Wfr_sb")
        nc.sync.dma_start(out=Wfr_sb, in_=Wfr_d[:, :, :])
        bexp_sb = cp.tile([E, D], BF16, name="bexp_sb")
        nc.sync.dma_start(out=bexp_sb, in_=bexp_d[:, :])
        ident_sb = cp.tile([128, 128], F32, name="ident_sb")
        nc.sync.dma_start(out=ident_sb, in_=ident_d[:, :])
        ones_r = cp.tile([128, 1], F32R, name="ones_r")
        nc.gpsimd.dma_start(out=ones_r, in_=ones_d[:, :])
        onesr_r = cp.tile([1, 128], F32R, name="onesr_r")
        nc.gpsimd.dma_start(out=onesr_r, in_=onesr_d[:, :])
        bg_bc = cp.tile([128, E], F32, name="bg_bc")
        nc.gpsimd.dma_start(
            out=bg_bc,
            in_=bass.AP(tensor=bg_d[:, :].tensor, offset=0, ap=[[0, 128], [1, E]]),
        )
        c0_sb = cp.tile([1, 1], F32, name="c0_sb")
        nc.sync.dma_start(out=c0_sb, in_=c0_d[:, :])

        def ldvec(dram, nch, name):
            t = cp.tile([128, nch], F32, name=name)
            nc.sync.dma_start(out=t, in_=dram[:, :])
            return t

        bp_sb = [ldvec(bp[i], _chunks(D), f"bp{i}_sb") for i in range(3)]
        sc1_sb = ldvec(sc1, _chunks(H), "sc1_sb")
        bi1_sb = ldvec(bi1, _chunks(H), "bi1_sb")
        sc2_sb = ldvec(sc2, _chunks(H), "sc2_sb")
        bi2_sb = ldvec(bi2, _chunks(H), "bi2_sb")
        b3_sb = ldvec(b3v, _chunks(D3), "b3_sb")
        lng_sb = ldvec(lngv, _chunks(D3), "lng_sb")
        lnb_sb = ldvec(lnbv, _chunks(D3), "lnb_sb")

        def split_act(src_ap):
            """tf32 hi/lo split of one [128, CS] fp32 activation chunk."""
            hi = pt([128, CS], F32R, tag="aph", bufs=2)
            nc.scalar.copy(hi, src_ap)
            lo = pt([128, CS], F32R, tag="apl", bufs=2)
            nc.vector.scalar_tensor_tensor(out=lo, in0=src_ap, scalar=-1.0,
                                           in1=hi.bitcast(F32), op0=ALU.bypass,
                                           op1=ALU.subtract)
            return hi, lo

        def load_w_pair(w_dram, k, dgs, dgw):
            """Load fp32 weight chunk [128, dgw] (k-chunk k, dout slice
            [dgs, dgs+dgw)) and split into tf32 hi/lo on chip."""
            wk = pt([128, 1024], F32, tag="wkf", bufs=3)[:, :dgw]
            nc.sync.dma_start(out=wk,
                              in_=w_dram[128 * k:128 * (k + 1), dgs:dgs + dgw])
            wh = pt([128, 1024], F32R, tag="wkh", bufs=2)[:, :dgw]
            nc.scalar.copy(wh, wk)
            wl = pt([128, 1024], F32R, tag="wkl", bufs=2)[:, :dgw]
            nc.vector.scalar_tensor_tensor(out=wl, in0=wk, scalar=-1.0,
                                           in1=wh.bitcast(F32), op0=ALU.bypass,
                                           op1=ALU.subtract)
            return wh, wl

        def mm3(psum, wh, wl, xh, xl, d, start, stop):
            sl = slice(128 * d, 128 * (d + 1))
            nc.tensor.matmul(psum, wh[:, sl], xh, start=start, stop=False)
            nc.tensor.matmul(psum, wh[:, sl], xl, start=False, stop=False)
            nc.tensor.matmul(psum, wl[:, sl], xh, start=False, stop=stop)

        # ---------------- megatile loop ----------------
        for t in range(T):
            ts = slice(t * CS, (t + 1) * CS)

            # ---- stage A: three projections -> comb [128, 12, CS] ----
            comb = pt([128, _chunks(D3), CS], F32, tag="big12", bufs=2)
            for i in range(3):
                psums = [mmtile(f"pp{i}") for _ in range(4)]
                for k in range(_chunks(DIN)):
                    xk = pt([128, CS], F32, tag="wkf", bufs=3)
                    nc.sync.dma_start(out=xk, in_=xs[i][128 * k:128 * (k + 1), ts])
                    xh, xl = split_act(xk)
                    wh, wl = load_w_pair(Wp[i], k, 0, 512)
                    for d in range(4):
                        mm3(psums[d], wh, wl, xh, xl, d,
                            start=(k == 0), stop=(k == _chunks(DIN) - 1))
                for d in range(4):
                    dd = 4 * i + d
                    nc.scalar.activation(comb[:, dd, :], psums[d], AF.Identity,
                                         bias=bp_sb[i][:, d:d + 1], scale=1.0)

            if t == 0:
                nc.sync.dma_start(out=We_sb, in_=We_d[:, :, :, :])

            # ---- W1 -> h1, W2 -> h2: single dout pass, 8 psum banks ----
            def dense_relu_bn(act_in, w_dram, kch, sc_sb, bi_sb):
                hout = pt([128, _chunks(H), CS], F32, tag="h", bufs=2)
                psums = [mmtile("ph") for _ in range(8)]
                for k in range(kch):
                    ah, al = split_act(act_in[:, k, :])
                    wh, wl = load_w_pair(w_dram, k, 0, 1024)
                    for d in range(8):
                        mm3(psums[d], wh, wl, ah, al, d,
                            start=(k == 0), stop=(k == kch - 1))
                for d in range(8):
                    nc.scalar.activation(hout[:, d, :], psums[d], AF.Relu,
                                         bias=bi_sb[:, d:d + 1],
                                         scale=sc_sb[:, d:d + 1])
                return hout

            h1 = dense_relu_bn(comb, W1, _chunks(D3), sc1_sb, bi1_sb)
            h2 = dense_relu_bn(h1, W2, _chunks(H), sc2_sb, bi2_sb)

            # ---- W3 -> t3 (+b3), two dout groups of 6; fp32r LN stats ----
            t3 = pt([128, _chunks(D3), CS], F32, tag="big12", bufs=2)
            psum_sum = ps.tile([1, CS], F32, tag="mm", name="psum_sum")
            psum_sq = ps.tile([1, CS], F32, tag="mm", name="psum_sq")
            for dg in range(2):
                psums = [mmtile("pw3") for _ in range(6)]
                for k in range(_chunks(H)):
                    ah, al = split_act(h2[:, k, :])
                    wh, wl = load_w_pair(W3, k, 768 * dg, 768)
                    for d in range(6):
                        mm3(psums[d], wh, wl, ah, al, d,
                            start=(k == 0), stop=(k == _chunks(H) - 1))
                for d in range(6):
                    dd = 6 * dg + d
                    nc.scalar.activation(t3[:, dd, :], psums[d], AF.Identity,
                                         bias=b3_sb[:, dd:dd + 1], scale=1.0)
                    t3r = pt([128, CS], F32R, tag="t3r", bufs=2)
                    nc.scalar.copy(t3r, t3[:, dd, :])
                    sqr = pt([128, CS], F32R, tag="sqr", bufs=2)
                    nc.scalar.activation(sqr, t3[:, dd, :], AF.Square)
                    nc.tensor.matmul(psum_sum, ones_r, t3r,
                                     start=(dd == 0), stop=(dd == _chunks(D3) - 1))
                    nc.tensor.matmul(psum_sq, ones_r, sqr,
                                     start=(dd == 0), stop=(dd == _chunks(D3) - 1))

            # ---- LN stats -> r, mean*r; broadcast across partitions ----
            msq = pt([1, CS], F32, tag="st1", bufs=4)
            nc.scalar.activation(msq, psum_sum, AF.Square, scale=1.0 / D3)
            e2p = pt([1, CS], F32, tag="st1", bufs=4)
            nc.scalar.activation(e2p, psum_sq, AF.Copy, bias=EPS, scale=1.0 / D3)
            mean_sb = pt([1, CS], F32, tag="st1", bufs=4)
            nc.scalar.activation(mean_sb, psum_sum, AF.Identity, scale=1.0 / D3)
            veps = pt([1, CS], F32, tag="st1", bufs=4)
            nc.vector.scalar_tensor_tensor(out=veps, in0=msq, scalar=-1.0, in1=e2p,
                                           op0=ALU.mult, op1=ALU.add)
            sdev = pt([1, CS], F32, tag="st1", bufs=4)
            nc.scalar.activation(sdev, veps, AF.Sqrt)
            r_sb = pt([1, CS], F32, tag="st1", bufs=4)
            nc.vector.reciprocal(r_sb, sdev)
            mr_sb = pt([1, CS], F32, tag="st1", bufs=4)
            nc.vector.tensor_mul(mr_sb, mean_sb, r_sb)
            r_r = pt([1, CS], F32R, tag="st1r", bufs=2)
            nc.scalar.copy(r_r, r_sb)
            mr_r = pt([1, CS], F32R, tag="st1r", bufs=2)
            nc.scalar.copy(mr_r, mr_sb)
            psum_rb = mmtile("psum_rb")
            nc.tensor.matmul(psum_rb, onesr_r, r_r, start=True, stop=True)
            rbc = pt([128, CS], F32, tag="bcast", bufs=2)
            nc.scalar.copy(rbc, psum_rb)
            psum_mrb = mmtile("psum_mrb")
            nc.tensor.matmul(psum_mrb, onesr_r, mr_r, start=True, stop=True)
            mrbc = pt([128, CS], F32, tag="bcast", bufs=2)
            nc.scalar.copy(mrbc, psum_mrb)

            # ---- normalize in place: t3 <- LN(t3) =: m ; bf16 copy ----
            m = t3
            for k in range(_chunks(D3)):
                nc.vector.tensor_mul(t3[:, k, :], t3[:, k, :], rbc)
                nc.vector.tensor_sub(t3[:, k, :], t3[:, k, :], mrbc)
                nc.scalar.activation(m[:, k, :], t3[:, k, :], AF.Identity,
                                     bias=lnb_sb[:, k:k + 1], scale=lng_sb[:, k:k + 1])

            # ---- MoE on the three parts + folded output accumulation ----
            outacc = pt([1, CS], F32, tag="outacc", bufs=1)
            for j in range(3):
                o_part = pt([128, 4, CS], BF16, tag="opart", bufs=2)
                mbfp = pt([128, 4, CS], BF16, tag="mbfp", bufs=2)
                for k in range(4):
                    nc.scalar.copy(mbfp[:, k, :], m[:, 4 * j + k, :])
                pend = None
                for c in range(NC_TOK):
                    cs_ = slice(128 * c, 128 * (c + 1))
                    # router logits (fp32, exact routing)
                    psum_log = ps.tile([128, E], F32, tag="mm", name="psum_log")
                    for k in range(4):
                        nc.tensor.matmul(psum_log, m[:, 4 * j + k, cs_],
                                         Wg_sb[:, k, :], start=(k == 0), stop=(k == 3))
                    logits = pt([128, E], F32, tag="logits", bufs=2)
                    nc.vector.tensor_add(logits, psum_log, bg_bc)
                    # top-2 + softmax weights per token
                    max1 = pt([128, 1], F32, tag="max1", bufs=2)
                    nc.vector.reduce_max(max1, logits, axis=AX.X)
                    is1 = pt([128, E], F32, tag="is1", bufs=2)
                    nc.vector.tensor_scalar(out=is1, in0=logits, scalar1=max1,
                                            scalar2=None, op0=ALU.is_equal)
                    l2 = pt([128, E], F32, tag="l2", bufs=2)
                    nc.vector.scalar_tensor_tensor(out=l2, in0=is1, scalar=NEG_BIG,
                                                   in1=logits, op0=ALU.mult,
                                                   op1=ALU.add)
                    max2 = pt([128, 1], F32, tag="max2", bufs=2)
                    nc.vector.reduce_max(max2, l2, axis=AX.X)
                    dlt = pt([128, 1], F32, tag="dlt", bufs=2)
                    nc.vector.tensor_sub(dlt, max1, max2)
                    s1 = pt([128, 1], F32, tag="s1", bufs=2)
                    nc.scalar.activation(s1, dlt, AF.Sigmoid)
                    s2 = pt([128, 1], F32, tag="s2", bufs=2)
                    nc.scalar.activation(s2, dlt, AF.Sigmoid, scale=-1.0)
                    is2 = pt([128, E], F32, tag="is2", bufs=2)
                    nc.vector.tensor_scalar(out=is2, in0=l2, scalar1=max2,
                                            scalar2=None, op0=ALU.is_equal)
                    w_sb = pt([128, E], F32, tag="w_sb", bufs=2)
                    nc.vector.tensor_scalar(out=w_sb, in0=is1, scalar1=s1,
                                            scalar2=None, op0=ALU.mult)
                    nc.vector.scalar_tensor_tensor(out=w_sb, in0=is2, scalar=s2,
                                                   in1=w_sb, op0=ALU.mult, op1=ALU.add)
                    # dense experts in four groups of 2, combined token-major;
                    # o_sb starts from the expert-bias term (w @ bexp)
                    o_sb = pt([128, CS], F32, tag="o_sb", bufs=2)
                    for g in range(4):
                        eps_ = [mmtile("pe") for _ in range(2)]
                        for k in range(4):
                            lhsT = mbfp[:, k, cs_]
                            for ei in range(2):
                                e = 2 * g + ei
                                nc.tensor.matmul(eps_[ei], lhsT, We_sb[:, e, k, :],
                                                 start=(k == 0), stop=(k == 3))
                        if g == 0:
                            # wT + bias matmul while DVE finishes w
                            psum_wt = ps.tile([E, 128], F32, tag="mm", name="psum_wt")
                            nc.tensor.transpose(psum_wt, w_sb, ident_sb)
                            wT_bf = pt([E, 128], BF16, tag="wT_bf", bufs=2)
                            nc.scalar.copy(wT_bf, psum_wt)
                            psum_b = mmtile("psum_b")
                            nc.tensor.matmul(psum_b, wT_bf, bexp_sb,
                                             start=True, stop=True)
                            nc.scalar.copy(o_sb, psum_b)
                        for ei in range(2):
                            e = 2 * g + ei
                            nc.vector.scalar_tensor_tensor(
                                out=o_sb, in0=eps_[ei], scalar=w_sb[:, e:e + 1],
                                in1=o_sb, op0=ALU.mult, op1=ALU.add)
                        if g == 1 and pend is not None:
                            # pipelined: transpose the PREVIOUS chunk's output
                            po, pc = pend
                            psum_ot = mmtile("psum_ot")
                            for d in range(4):
                                nc.tensor.transpose(
                                    psum_ot[:, 128 * d:128 * (d + 1)],
                                    po[:, 128 * d:128 * (d + 1)], ident_sb)
                            nc.scalar.copy(
                                o_part[:, :, slice(128 * pc, 128 * (pc + 1))],
                                psum_ot.rearrange("p (d c) -> p d c", d=4))
                            pend = None
                    pend = (o_sb, c)
                # drain the last chunk's transpose
                po, pc = pend
                psum_ot = mmtile("psum_ot")
                for d in range(4):
                    nc.tensor.transpose(psum_ot[:, 128 * d:128 * (d + 1)],
                                        po[:, 128 * d:128 * (d + 1)], ident_sb)
                nc.scalar.copy(o_part[:, :, slice(128 * pc, 128 * (pc + 1))],
                               psum_ot.rearrange("p (d c) -> p d c", d=4))
                # folded output: outacc += o_part @ Wfr[part j]
                psum_oj = ps.tile([1, CS], F32, tag="mm", name="psum_oj")
                for k in range(4):
                    nc.tensor.matmul(psum_oj, Wfr_sb[:, 4 * j + k, :],
                                     o_part[:, k, :], start=(k == 0), stop=(k == 3))
                if j == 0:
                    nc.scalar.copy(outacc, psum_oj)
                else:
                    nc.vector.tensor_add(outacc, outacc, psum_oj)
            orow = pt([1, CS], F32, tag="orow", bufs=1)
            nc.scalar.activation(orow, outacc, AF.Identity, bias=c0_sb, scale=1.0)
            nc.sync.dma_start(out=out_d[:, ts], in_=orow)

    nc.compile()
    return nc


def _pack_vec(v, nch):
    return np.ascontiguousarray(v.reshape(nch, 128).T.astype(np.float32))


def _tf32_split(w):
    """Split fp32 matrix into tf32-representable hi + lo (RNE at 11
    mantissa bits, matching the PE's fp32r rounding)."""
    w = np.ascontiguousarray(w, np.float32)

    def rnd(x):
        u = x.view(np.uint32)
        keep = ((u + 0x800 + ((u >> 12) & 1)) & 0xFFFFF000).astype(np.uint32)
        return keep.view(np.float32)

    hi = rnd(w)
    lo = rnd((w.astype(np.float64) - hi.astype(np.float64)).astype(np.float32))
    return hi, lo


def prepare_maps(inputs):
    """Host-side sharding + weight prep. Returns per-core input maps."""
    f32 = np.float32
    k64 = 1.0 / np.sqrt(np.float64(1.0) + np.float64(EPS))
    k = f32(k64)
    g1 = inputs["g1"].astype(f32)
    g2 = inputs["g2"].astype(f32)
    # folded output vector: out = concat(o) @ (Wf @ (scf*Wr)) + c0
    scf64 = inputs["bng"].astype(np.float64) * k64
    wfr64 = inputs["Wf"].astype(np.float64) @ (scf64 * inputs["Wr"][:, 0].astype(np.float64))
    c064 = (float(np.dot(inputs["bf"].astype(np.float64) * scf64
                         + inputs["bnb"].astype(np.float64),
                         inputs["Wr"][:, 0].astype(np.float64)))
            + float(inputs["br"][0]))
    consts = {
        "sc1": _pack_vec(g1 * k, _chunks(H)),
        "bi1": _pack_vec(inputs["b1"] * g1 * k + inputs["be1"], _chunks(H)),
        "sc2": _pack_vec(g2 * k, _chunks(H)),
        "bi2": _pack_vec(inputs["b2"] * g2 * k + inputs["be2"], _chunks(H)),
        "b3v": _pack_vec(inputs["b3"], _chunks(D3)),
        "lngv": _pack_vec(inputs["lng"], _chunks(D3)),
        "lnbv": _pack_vec(inputs["lnb"], _chunks(D3)),
        "Wg_r": np.ascontiguousarray(
            inputs["Wg"].reshape(_chunks(D), 128, E).transpose(1, 0, 2), f32),
        "Wfr_r": np.ascontiguousarray(
            wfr64.astype(f32).reshape(_chunks(D3), 128, 1).transpose(1, 0, 2)
        ).astype(ml_dtypes.bfloat16),
        "We_r": np.ascontiguousarray(
            inputs["We"].reshape(E, _chunks(D), 128, D).transpose(2, 0, 1, 3)
        ).astype(ml_dtypes.bfloat16),
        "bexp_bf": np.ascontiguousarray(inputs["bexp"]).astype(ml_dtypes.bfloat16),
        "bg_v": np.ascontiguousarray(inputs["bg"], f32).reshape(1, E),
        "c0_v": np.full((1, 1), c064, f32),
        "ones_col": np.ones((128, 1), f32),
        "ones_row": np.ones((1, 128), f32),
        "ident": np.eye(128, dtype=f32),
    }
    for nm in ["W1", "W2", "W3"]:
        consts[nm] = np.ascontiguousarray(inputs[nm], f32)
    for i in range(3):
        consts[f"Wp{i+1}"] = np.ascontiguousarray(inputs[f"Wp{i+1}"], f32)
        consts[f"bp{i+1}"] = _pack_vec(inputs[f"bp{i+1}"], _chunks(D))
    xts = [np.ascontiguousarray(inputs[f"x{i+1}"].astype(f32).T) for i in range(3)]
    in_maps = []
    for c in range(N_CORES):
        m = dict(consts)
        sl = slice(c * TOK_CORE, (c + 1) * TOK_CORE)
        for i in range(3):
            m[f"x{i+1}t"] = np.ascontiguousarray(xts[i][:, sl])
        in_maps.append(m)
    return in_maps


def run(inputs, trace=False, n_tok=TOK_CORE):
    key = n_tok
    if key not in _PROGRAM_CACHE:
        _PROGRAM_CACHE[key] = build_program(n_tok=n_tok)
    nc = _PROGRAM_CACHE[key]
    in_maps = prepare_maps(inputs)
    res = run_bass_kernel_spmd(nc, in_maps, list(range(N_CORES)), trace=trace)
    rows = [res.results[c]["out"][0] for c in range(N_CORES)]
    out = np.concatenate(rows).reshape(B, 1).astype(np.float32)
    return out, res


def kernel(**inputs):
    out, _ = run(inputs, trace=False)
    return out

